# revision 24
# baseline (speedup 1.0000x reference)
"""Distributed multi-head attention for Trainium2 (8 NeuronCores).

Problem: nn_MultiHeadAttention (B=2, S=2048, D=1024, H=16, HD=64), f32.

Sharding: tensor parallel over heads — core c owns heads {2c, 2c+1}
(feature slice [128c, 128c+128)) and processes BOTH batches for them.
The output projection is sequence-parallel: eight 8-core AllToAlls
(one per head-parity x batch x token-half) exchange 128-token chunks
of the per-head attention outputs, after which core c holds all 16
heads for tokens {128c..128c+128} and {1024+128c..} of EACH batch and
contracts the full 1024 attention features against Wo. Fine-grained
collectives keep the exchanges off the tail: only the last 128KB
exchange and a 128-token projection remain serial.

Matmuls run in bf16 (f32 PSUM accumulate). Key Trainium2 facts shaping
the implementation (HW-measured here):
  - K=64 matmuls stream at ~2 cyc/col vs 1 for K=128, so the scores
    matmuls use per-head zero-padded KT tiles (K=128, zeros kill the
    other head's contribution; QT needs no masking).
  - The Tile scheduler is priority-list based (emission order is the
    priority). ScalarE exp (~140us busy) is co-critical with PE
    (~200us): emission order feeds the exp stream as early as the
    DMA ramp allows (K0/Q0/V0 then attention; remaining projections
    fill PE idle slots between ACT-paced score matmuls).
  - DMA has ~12us fixed startup and ~190GB/s practical aggregate; a
    1MB x block takes ~11us. x is host-relaid to [p, blk, e, n] so
    blocks load with 8KB-contiguous lines; batch-1 x prefetches during
    batch-0 attention so the mid-kernel collectives don't contend.
  - ScalarE does ONLY exp (switching activation functions reloads
    LUTs); all PSUM evacuations go through VectorE with fused bias.
  - exp is done on [128, 1024] tiles (2 PSUM banks) to amortize ~250ns
    of per-instruction ACT overhead.
  - attn^T = V_aug.T @ exp accumulated over k tiles, where V_aug
    carries a ones column -> psum row 64 is the softmax denominator.
  - 1/den uses reciprocal_approx_fast (~18 bits, 5x faster than the
    exact DVE reciprocal that cost 3.3us per tile).
  - No max subtraction in softmax: scores ~ N(0,1) by construction.
"""

import numpy as np

B = 2
S = 2048          # both n_q and k (per batch)
TS = B * S        # combined token axis (4096)
D = 1024          # embed dim
H = 16            # heads
HD = 64           # head dim
N_CORES = 8
GH = 2            # heads per core
GF = GH * HD      # 128 per-core head features
TB = 512          # token block (projection / attention q block)
OC = 128          # owned tokens per (batch, half) per core (a2a chunk)
NBLK = S // TB    # 4 token blocks per batch
NKT = S // 128    # 16 k tiles per batch
ECH = D // 128    # 8 contraction chunks of the embed dim
NP = NKT // 2     # 8 k-tile pairs (wide 1024-col exp tiles)

_CACHE = {}
MM_DTYPE = "bf16"  # "bf16" or "f32r"


def _build():
    import concourse.bacc as bacc
    import concourse.tile as tile
    from concourse import mybir

    F32 = mybir.dt.float32
    MDT = mybir.dt.bfloat16 if MM_DTYPE == "bf16" else mybir.dt.float32r
    Act = mybir.ActivationFunctionType

    nc = bacc.Bacc("TRN2", target_bir_lowering=False, debug=False,
                   num_devices=N_CORES)

    # ---- kernel I/O ----
    # x layout: [p, blk, e, n] -> per-partition 8KB contiguous per block
    xq_d = nc.dram_tensor("xq", [128, B * NBLK, ECH, TB], MDT,
                          kind="ExternalInput")
    xk_d = nc.dram_tensor("xk", [128, B * NBLK, ECH, TB], MDT,
                          kind="ExternalInput")
    xv_d = nc.dram_tensor("xv", [128, B * NBLK, ECH, TB], MDT,
                          kind="ExternalInput")
    wqT = nc.dram_tensor("wqT", [128, ECH, GF], MDT, kind="ExternalInput")
    wkT = nc.dram_tensor("wkT", [128, ECH, GF], MDT, kind="ExternalInput")
    wvT = nc.dram_tensor("wvT", [128, ECH, GF], MDT, kind="ExternalInput")
    woT = nc.dram_tensor("woT", [128, ECH, D], MDT, kind="ExternalInput")

    bq_d = nc.dram_tensor("bq", [128, 1], F32, kind="ExternalInput")
    bk_d = nc.dram_tensor("bk", [128, 1], F32, kind="ExternalInput")
    kmask_d = nc.dram_tensor("kmask", [128, GH], F32, kind="ExternalInput")
    kbm_d = nc.dram_tensor("kbm", [128, GH], F32, kind="ExternalInput")
    bv_d = nc.dram_tensor("bv", [128, TB], F32, kind="ExternalInput")
    bo_d = nc.dram_tensor("bo", [128, D], F32, kind="ExternalInput")
    out_d = nc.dram_tensor("out", [B, 2, 128, D], F32, kind="ExternalOutput")

    groups = [list(range(N_CORES))]

    with tile.TileContext(nc) as tc:
        with (
            tc.tile_pool(name="wpool", bufs=1) as wpool,
            tc.tile_pool(name="state", bufs=1) as state,
            tc.tile_pool(name="xpool", bufs=4) as xpool,
            tc.tile_pool(name="expp", bufs=4) as expp,
            tc.tile_pool(name="small", bufs=3) as small,
            tc.tile_pool(name="opool", bufs=2) as opool,
            tc.tile_pool(name="ps_proj", bufs=2, space="PSUM") as ps_proj,
            tc.tile_pool(name="ps_sc", bufs=2, space="PSUM") as ps_sc,
            tc.tile_pool(name="ps_at", bufs=2, space="PSUM") as ps_at,
            tc.tile_pool(name="dramp", bufs=1, space="DRAM") as dramp,
        ):
            # ---- startup DMAs ----
            # gpsimd: weights first (K proj starts the PE stream), then
            # the small constants. sync/scalar: the two exp-critical x
            # blocks first, one per engine.
            wk_sb = wpool.tile([128, ECH, GF], MDT, name="wk_sb")
            nc.gpsimd.dma_start(wk_sb[:], wkT[:])
            wq_sb = wpool.tile([128, ECH, GF], MDT, name="wq_sb")
            nc.gpsimd.dma_start(wq_sb[:], wqT[:])
            wv_sb = wpool.tile([128, ECH, GF], MDT, name="wv_sb")
            nc.gpsimd.dma_start(wv_sb[:], wvT[:])
            # small constants on scalar: they finish before the first exp,
            # after which the scalar queue must stay DMA-free (descriptor
            # generation on the ACT sequencer slows every exp dispatch).
            bq_sb = wpool.tile([128, 1], F32, name="bq_sb")
            nc.scalar.dma_start(bq_sb[:], bq_d[:])
            kmask_sb = wpool.tile([128, GH], F32, name="kmask_sb")
            nc.scalar.dma_start(kmask_sb[:], kmask_d[:])
            kbm_sb = wpool.tile([128, GH], F32, name="kbm_sb")
            nc.scalar.dma_start(kbm_sb[:], kbm_d[:])
            bv_sb = wpool.tile([128, TB], F32, name="bv_sb")
            nc.scalar.dma_start(bv_sb[:], bv_d[:])

            xt = {}  # (name, b, t) -> sbuf tile (or [halfA, halfB])

            def load_x(name, b, t, eng, split=False):
                src = {"q": xq_d, "k": xk_d, "v": xv_d}[name]
                blk = b * NBLK + t
                if split:
                    # two e-halves so the projection can start on the
                    # first 4 contraction chunks while the rest lands
                    halves = []
                    for i in range(2):
                        tl = xpool.tile([128, ECH // 2, TB], MDT, tag="x",
                                        name=f"x{name}{b}{t}_{i}")
                        eng.dma_start(tl[:], src[:, blk, 4 * i:4 * i + 4])
                        halves.append(tl)
                    xt[(name, b, t)] = halves
                else:
                    tl = xpool.tile([128, ECH, TB], MDT, tag="x",
                                    name=f"x{name}{b}{t}")
                    eng.dma_start(tl[:], src[:, blk])
                    xt[(name, b, t)] = tl

            def xchunk(name, b, t, e):
                tl = xt[(name, b, t)]
                if isinstance(tl, list):
                    return tl[e // 4][:, e % 4, :]
                return tl[:, e, :]

            # DMA behavior (HW-measured): each engine's transfers run
            # roughly serially in issue order; concurrent transfers
            # fair-share the aggregate. So the whole exp-critical
            # sequence (K+Q, interleaved in consumption order, block-0
            # tiles halved) goes on the fast sync column; V rides
            # scalar/gpsimd in need order behind the weights.
            load_x("k", 0, 0, nc.sync, split=True)
            load_x("q", 0, 0, nc.sync, split=True)
            load_x("v", 0, 0, nc.gpsimd)
            for t in range(1, NBLK):
                load_x("k", 0, t, nc.sync)
                load_x("q", 0, t, nc.sync)
                load_x("v", 0, t, nc.scalar if t == 1 else nc.gpsimd)

            # ---- long-lived state ----
            QT = state.tile([128, TS], MDT, name="QT")
            AT = state.tile([128, TS], MDT, name="AT")
            # per-head zero-padded KT: rows [64h, 64h+64) hold head h's
            # K features, the other 64 rows stay zero -> scores matmuls
            # run K=128 (2x faster than K=64) with unmasked QT as rhs.
            KTp = [state.tile([128, TS], MDT, name=f"KTp{h}")
                   for h in range(GH)]

            # V: [128 tok, tok-chunk, head, 65]; col 64 = ones
            VT = state.tile([128, B * NKT, GH, HD + 1], MDT, name="VT")
            nc.gpsimd.memset(VT[:, :, :, HD:HD + 1], 1.0)

            ao = [state.tile([128, ECH, 2 * OC], MDT, name=f"ao{b}")
                  for b in range(B)]
            # batch 0: one exchange per 1024-token half (128-token chunks)
            # batch 1: one exchange per 512-token q-block (64-token
            # chunks) so the LAST exchange launches right after qb3 and
            # carries only 64KB. ao column layouts match per batch.
            a2a_in = {}
            a2a_out = {}
            for h in range(GH):
                for hf in range(2):
                    a2a_in[(h, 0, hf)] = dramp.tile(
                        [N_CORES, HD, OC], MDT, name=f"a2a_in{h}0{hf}")
                    a2a_out[(h, 0, hf)] = dramp.tile(
                        [N_CORES, HD, OC], MDT, name=f"a2a_out{h}0{hf}")
                for qb in range(NBLK):
                    a2a_in[(h, 1, qb)] = dramp.tile(
                        [N_CORES, HD, OC // 2], MDT, name=f"a2a_in{h}1q{qb}")
                    a2a_out[(h, 1, qb)] = dramp.tile(
                        [N_CORES, HD, OC // 2], MDT, name=f"a2a_out{h}1q{qb}")

            # ---- emission helpers ----
            def emit_qk_proj(name, b, t):
                w_sb = wq_sb if name == "q" else wk_sb
                col = b * S + t * TB
                csl = slice(col, col + TB)
                ps = ps_proj.tile([128, TB], F32, tag="pp",
                                  name=f"ps{name}{b}{t}")
                for e in range(ECH):
                    nc.tensor.matmul(ps[:], w_sb[:, e, :],
                                     xchunk(name, b, t, e),
                                     start=(e == 0), stop=(e == ECH - 1))
                if name == "q":
                    nc.vector.tensor_scalar_add(QT[:, csl], ps[:], bq_sb[:])
                else:
                    for h in range(GH):
                        nc.vector.tensor_scalar(
                            KTp[h][:, csl], ps[:],
                            kmask_sb[:, h:h + 1], kbm_sb[:, h:h + 1],
                            op0=mybir.AluOpType.mult,
                            op1=mybir.AluOpType.add)

            def emit_v_proj(b, t):
                psv = ps_proj.tile([128, TB], F32, tag="pp",
                                   name=f"psv{b}{t}")
                for e in range(ECH):
                    for m in range(4):
                        # NOTE: start=True clears has_written for the
                        # WHOLE psum bank, so only the very first matmul
                        # into this bank may set it.
                        nc.tensor.matmul(
                            psv[:, m * GF:(m + 1) * GF],
                            xchunk("v", b, t, e)[:, m * 128:(m + 1) * 128],
                            wv_sb[:, e, :],
                            start=(e == 0 and m == 0),
                            stop=(e == ECH - 1 and m == 3))
                kt0 = b * NKT + t * 4
                nc.vector.tensor_add(
                    VT[:, kt0:kt0 + 4, :, 0:HD],
                    psv[:].rearrange("p (m h d) -> p m h d", m=4, h=GH),
                    bv_sb[:].rearrange("p (m h d) -> p m h d", m=4, h=GH))

            def emit_attn(h, b, stage_eng):
                """Attention for head-parity h, batch b. After each
                1024-token half: stage + AllToAll + receive into ao."""
                off = HD * h
                for qb in range(NBLK):
                    qcol = b * S + qb * TB
                    qsl = slice(qcol, qcol + TB)
                    pa = ps_at.tile([HD + 1, TB], F32, tag="at",
                                    name=f"pa{h}{b}{qb}")
                    exps = []
                    for kp in range(NP):
                        pssc = ps_sc.tile([128, 2 * TB], F32, tag="sc",
                                          name=f"pssc{h}{b}{qb}{kp}")
                        for i in range(2):
                            kcol = b * S + (2 * kp + i) * 128
                            nc.tensor.matmul(
                                pssc[:, i * TB:(i + 1) * TB],
                                KTp[h][:, kcol:kcol + 128],
                                QT[:, qsl], start=True, stop=True)
                        ex = expp.tile([128, 2 * TB], MDT, tag="exp",
                                       name=f"ex{h}{b}{qb}{kp}")
                        nc.scalar.activation(ex[:], pssc[:], Act.Exp,
                                             scale=0.125)
                        exps.append(ex)
                        if kp >= 1:
                            for i in range(2):
                                kt = 2 * (kp - 1) + i
                                nc.tensor.matmul(
                                    pa[:],
                                    VT[:, b * NKT + kt, h, :],
                                    exps[kp - 1][:, i * TB:(i + 1) * TB],
                                    start=(kt == 0), stop=False)
                    for i in range(2):
                        kt = 2 * (NP - 1) + i
                        nc.tensor.matmul(
                            pa[:], VT[:, b * NKT + kt, h, :],
                            exps[NP - 1][:, i * TB:(i + 1) * TB],
                            start=False, stop=(i == 1))
                    # normalize: attnT_h *= 1/den (broadcast over d)
                    dn = small.tile([1, TB], F32, tag="rc",
                                    name=f"dn{h}{b}{qb}")
                    nc.vector.tensor_copy(dn[:], pa[HD:HD + 1, :])
                    rc1 = small.tile([1, TB], F32, tag="rc1",
                                     name=f"rc1{h}{b}{qb}")
                    nc.vector.reciprocal_approx_fast(rc1[:], dn[:])
                    rc = small.tile([HD, TB], F32, tag="rc2",
                                    name=f"rc{h}{b}{qb}")
                    nc.gpsimd.partition_broadcast(rc[:], rc1[:])
                    nc.vector.tensor_mul(
                        AT[off:off + HD, qsl], pa[0:HD, :], rc[:])
                    if b == 1:
                        key, w = (h, 1, qb), OC // 2
                        src = AT[off:off + HD, qsl]
                        dst = ao[1][off:off + HD, :, w * qb:w * (qb + 1)]
                    elif qb % 2 == 1:
                        hf = qb // 2
                        key, w = (h, 0, hf), OC
                        src = AT[off:off + HD,
                                 1024 * hf:1024 * (hf + 1)]
                        dst = ao[0][off:off + HD, :, w * hf:w * (hf + 1)]
                    else:
                        continue
                    stage_eng.dma_start(
                        a2a_in[key][:].rearrange("j p n -> p j n"),
                        src.rearrange("p (j n) -> p j n", j=N_CORES))
                    nc.gpsimd.collective_compute(
                        "AllToAll",
                        mybir.AluOpType.bypass,
                        replica_groups=groups,
                        ins=[a2a_in[key][:]],
                        outs=[a2a_out[key][:]],
                    )
                    stage_eng.dma_start(
                        dst, a2a_out[key][:].rearrange("j p n -> p j n"))

            def emit_out_proj(b, m, eng):
                ot = opool.tile([128, D], F32, tag="ot", name=f"ot{b}{m}")
                for fb in range(2):
                    fsl = slice(fb * 512, (fb + 1) * 512)
                    pso = ps_proj.tile([128, 512], F32, tag="pp",
                                       name=f"pso{b}{m}{fb}")
                    for e in range(ECH):
                        nc.tensor.matmul(
                            pso[:], ao[b][:, e, m * OC:(m + 1) * OC],
                            wo_sb[:, e, fsl],
                            start=(e == 0), stop=(e == ECH - 1))
                    nc.vector.tensor_add(ot[:, fsl], pso[:], bo_sb[:, fsl])
                    # per-half store so the last 256KB isn't serialized
                    # behind the second half's matmuls
                    eng.dma_start(out_d[b, m, :, fsl], ot[:, fsl])

            # ---- schedule ----
            # Emission order defines BOTH the RAW dependencies (a reader
            # must be emitted after its writer: the tracker doesn't know
            # about future writes) and the scheduler priority. So all of
            # a batch's projections are emitted before its attention, and
            # the attention is wrapped in high_priority() so its scores
            # preempt remaining projection work once their deps resolve
            # (keeps the ScalarE exp stream dense).
            for t in range(NBLK):
                emit_qk_proj("k", 0, t)
                emit_qk_proj("q", 0, t)
                emit_v_proj(0, t)

            # wo/bo are needed only from ~150us; they ride the sync column
            # behind the batch-0 K/Q loads.
            wo_sb = wpool.tile([128, ECH, D], MDT, name="wo_sb")
            nc.sync.dma_start(wo_sb[:], woT[:])
            bo_sb = wpool.tile([128, D], F32, name="bo_sb")
            nc.sync.dma_start(bo_sb[:], bo_d[:])

            with tc.high_priority():
                emit_attn(0, 0, nc.sync)
            emit_attn(1, 0, nc.gpsimd)

            # batch-1 x prefetch + projections (fill PE under batch-0
            # attention, ahead of the mid-kernel collectives)
            for t in range(NBLK):
                load_x("k", 1, t, nc.sync)
                load_x("q", 1, t, nc.sync)
                load_x("v", 1, t, nc.gpsimd)
            emit_qk_proj("k", 1, 0)
            emit_qk_proj("q", 1, 0)
            emit_v_proj(1, 0)
            for t in range(1, NBLK):
                emit_qk_proj("k", 1, t)
                emit_qk_proj("q", 1, t)
                emit_v_proj(1, t)

            # offset ~= instruction count of the batch-1 projections, so
            # batch-1 scores outrank leftover projection work but stay
            # after the batch-0 attention priorities.
            with tc.high_priority(offset=400):
                emit_attn(0, 1, nc.sync)
            emit_out_proj(0, 0, nc.sync)   # overlaps batch-1 attention
            emit_out_proj(0, 1, nc.sync)
            with tc.high_priority(offset=450):
                emit_attn(1, 1, nc.gpsimd)
            emit_out_proj(1, 0, nc.gpsimd)  # ready mid-phase: keeps PE warm
            # discarded matmuls: hold the PE p-state up while the last
            # AllToAll is in flight so the final projection runs at full
            # clock (idle >0.1us drops PE to the 1.2GHz mid state)
            warm = ps_at.tile([HD + 1, TB], F32, tag="at", name="warm")
            for _ in range(24):
                nc.tensor.matmul(warm[:], VT[:, 0, 0, :], QT[:, 0:TB],
                                 start=True, stop=True)
            emit_out_proj(1, 1, nc.gpsimd)  # the only tail work

    nc.compile()
    return nc


def _mm_np_dtype():
    if MM_DTYPE == "bf16":
        import ml_dtypes
        return np.dtype(ml_dtypes.bfloat16)
    return np.float32


def _prep_inputs(Q_input, K_input, V_input, Wq, bq, Wk, bk, Wv, bv, Wo, bo):
    """Build the 8 per-core input maps (host-side sharding + transposes)."""
    f32 = np.float32
    mmdt = _mm_np_dtype()
    xT = {}
    for nm, x in (("xq", Q_input), ("xk", K_input), ("xv", V_input)):
        x = np.asarray(x, f32).reshape(TS, D)
        # [tok, feat] -> [p, blk, e, n] with tok = blk*TB + n,
        # feat = e*128 + p  (8KB contiguous per (p, blk))
        xT[nm] = np.ascontiguousarray(
            x.reshape(B * NBLK, TB, ECH, 128).transpose(3, 0, 2, 1)
            .astype(mmdt))
    Wq, Wk, Wv, Wo = (np.asarray(w, f32) for w in (Wq, Wk, Wv, Wo))
    bq, bk, bv, bo = (np.asarray(v, f32) for v in (bq, bk, bv, bo))

    def peF(wT):  # [D, F] -> [128, ECH, F] partition-major (fat descriptors)
        return np.ascontiguousarray(
            wT.reshape(ECH, 128, wT.shape[1]).transpose(1, 0, 2).astype(mmdt))

    woT_full = peF(Wo.T)
    bo_bc = np.ascontiguousarray(np.broadcast_to(bo, (128, D)))
    kmask = np.zeros((128, GH), f32)
    for h in range(GH):
        kmask[HD * h:HD * h + HD, h] = 1.0

    in_maps = []
    for c in range(N_CORES):
        hsl = slice(c * GF, (c + 1) * GF)
        in_maps.append({
            **xT,
            "wqT": peF(Wq[hsl, :].T),
            "wkT": peF(Wk[hsl, :].T),
            "wvT": peF(Wv[hsl, :].T),
            "woT": woT_full,
            "bq": np.ascontiguousarray(bq[hsl].reshape(128, 1)),
            "bk": np.ascontiguousarray(bk[hsl].reshape(128, 1)),
            "kmask": kmask,
            "kbm": np.ascontiguousarray(kmask * bk[hsl].reshape(128, 1)),
            "bv": np.ascontiguousarray(
                np.broadcast_to(np.tile(bv[hsl], 4), (128, TB))),
            "bo": bo_bc,
        })
    return in_maps


def kernel(**inputs):
    from concourse.bass_utils import run_bass_kernel_spmd

    if "nc" not in _CACHE:
        _CACHE["nc"] = _build()
    nc = _CACHE["nc"]

    in_maps = _prep_inputs(**inputs)
    res = run_bass_kernel_spmd(nc, in_maps, core_ids=list(range(N_CORES)))

    out = np.empty((B, S, D), np.float32)
    for c in range(N_CORES):
        o = res.results[c]["out"]  # [B, 2, 128, D]
        # batch 0: m-block m holds tokens 1024*m + 128*c + [0,128)
        for m in range(2):
            t0 = 1024 * m + OC * c
            out[0, t0:t0 + OC, :] = o[0, m]
        # batch 1: m-block m rows r -> qb = 2m + r//64, n = r % 64,
        # token = 512*qb + 64*c + n
        for m in range(2):
            for j in range(2):
                qb = 2 * m + j
                t0 = 512 * qb + 64 * c
                out[1, t0:t0 + 64, :] = o[1, m, 64 * j:64 * (j + 1)]
    return out


# revision 28
# speedup vs baseline: 1.0809x; 1.0809x over previous
"""Distributed multi-head attention for Trainium2 (8 NeuronCores).

Problem: nn_MultiHeadAttention (B=2, S=2048, D=1024, H=16, HD=64), f32.

Sharding: tensor parallel over heads — core c owns heads {2c, 2c+1}
(feature slice [128c, 128c+128)) and processes BOTH batches for them.
The output projection is sequence-parallel: eight 8-core AllToAlls
(one per head-parity x batch x token-half) exchange 128-token chunks
of the per-head attention outputs, after which core c holds all 16
heads for tokens {128c..128c+128} and {1024+128c..} of EACH batch and
contracts the full 1024 attention features against Wo. Fine-grained
collectives keep the exchanges off the tail: only the last 128KB
exchange and a 128-token projection remain serial.

Matmuls run in bf16 (f32 PSUM accumulate). Key Trainium2 facts shaping
the implementation (HW-measured here):
  - K=64 matmuls stream at ~2 cyc/col vs 1 for K=128, so the scores
    matmuls use per-head zero-padded KT tiles (K=128, zeros kill the
    other head's contribution; QT needs no masking).
  - The Tile scheduler is priority-list based (emission order is the
    priority). ScalarE exp (~140us busy) is co-critical with PE
    (~200us): emission order feeds the exp stream as early as the
    DMA ramp allows (K0/Q0/V0 then attention; remaining projections
    fill PE idle slots between ACT-paced score matmuls).
  - DMA has ~12us fixed startup and ~190GB/s practical aggregate; a
    1MB x block takes ~11us. x is host-relaid to [p, blk, e, n] so
    blocks load with 8KB-contiguous lines; batch-1 x prefetches during
    batch-0 attention so the mid-kernel collectives don't contend.
  - ScalarE does ONLY exp (switching activation functions reloads
    LUTs); all PSUM evacuations go through VectorE with fused bias.
  - exp is done on [128, 1024] tiles (2 PSUM banks) to amortize ~250ns
    of per-instruction ACT overhead.
  - attn^T = V_aug.T @ exp accumulated over k tiles, where V_aug
    carries a ones column -> psum row 64 is the softmax denominator.
  - 1/den uses reciprocal_approx_fast (~18 bits, 5x faster than the
    exact DVE reciprocal that cost 3.3us per tile).
  - No max subtraction in softmax: scores ~ N(0,1) by construction.
"""

import numpy as np

B = 2
S = 2048          # both n_q and k (per batch)
TS = B * S        # combined token axis (4096)
D = 1024          # embed dim
H = 16            # heads
HD = 64           # head dim
N_CORES = 8
GH = 2            # heads per core
GF = GH * HD      # 128 per-core head features
TB = 512          # token block (projection / attention q block)
OC = 128          # owned tokens per (batch, half) per core (a2a chunk)
NBLK = S // TB    # 4 token blocks per batch
NKT = S // 128    # 16 k tiles per batch
ECH = D // 128    # 8 contraction chunks of the embed dim
NP = NKT // 2     # 8 k-tile pairs (wide 1024-col exp tiles)

_CACHE = {}
MM_DTYPE = "bf16"  # "bf16" or "f32r"


def _build():
    import concourse.bacc as bacc
    import concourse.tile as tile
    from concourse import mybir

    F32 = mybir.dt.float32
    MDT = mybir.dt.bfloat16 if MM_DTYPE == "bf16" else mybir.dt.float32r
    Act = mybir.ActivationFunctionType

    nc = bacc.Bacc("TRN2", target_bir_lowering=False, debug=False,
                   num_devices=N_CORES)

    # ---- kernel I/O ----
    # x layout: [p, blk, e, n] -> per-partition 8KB contiguous per block
    xq_d = nc.dram_tensor("xq", [128, B * NBLK, ECH, TB], MDT,
                          kind="ExternalInput")
    xk_d = nc.dram_tensor("xk", [128, B * NBLK, ECH, TB], MDT,
                          kind="ExternalInput")
    xv_d = nc.dram_tensor("xv", [128, B * NBLK, ECH, TB], MDT,
                          kind="ExternalInput")
    wqT = nc.dram_tensor("wqT", [128, ECH, GF], MDT, kind="ExternalInput")
    wkT = nc.dram_tensor("wkT", [128, ECH, GF], MDT, kind="ExternalInput")
    wvT = nc.dram_tensor("wvT", [128, ECH, GF], MDT, kind="ExternalInput")
    woT = nc.dram_tensor("woT", [128, ECH, D], MDT, kind="ExternalInput")

    bq_d = nc.dram_tensor("bq", [128, 1], F32, kind="ExternalInput")
    bk_d = nc.dram_tensor("bk", [128, 1], F32, kind="ExternalInput")
    kmask_d = nc.dram_tensor("kmask", [128, GH], F32, kind="ExternalInput")
    kbm_d = nc.dram_tensor("kbm", [128, GH], F32, kind="ExternalInput")
    bv_d = nc.dram_tensor("bv", [128, TB], F32, kind="ExternalInput")
    bo_d = nc.dram_tensor("bo", [128, D], F32, kind="ExternalInput")
    out_d = nc.dram_tensor("out", [B, 2, 128, D], F32, kind="ExternalOutput")

    groups = [list(range(N_CORES))]

    with tile.TileContext(nc) as tc:
        with (
            tc.tile_pool(name="wpool", bufs=1) as wpool,
            tc.tile_pool(name="state", bufs=1) as state,
            tc.tile_pool(name="xpool", bufs=4) as xpool,
            tc.tile_pool(name="expp", bufs=6) as expp,
            tc.tile_pool(name="small", bufs=3) as small,
            tc.tile_pool(name="opool", bufs=2) as opool,
            tc.tile_pool(name="ps_proj", bufs=2, space="PSUM") as ps_proj,
            tc.tile_pool(name="ps_sc", bufs=2, space="PSUM") as ps_sc,
            tc.tile_pool(name="ps_at", bufs=2, space="PSUM") as ps_at,
            tc.tile_pool(name="dramp", bufs=1, space="DRAM") as dramp,
        ):
            # ---- startup DMAs ----
            # gpsimd: weights first (K proj starts the PE stream), then
            # the small constants. sync/scalar: the two exp-critical x
            # blocks first, one per engine.
            wk_sb = wpool.tile([128, ECH, GF], MDT, name="wk_sb")
            nc.gpsimd.dma_start(wk_sb[:], wkT[:])
            wq_sb = wpool.tile([128, ECH, GF], MDT, name="wq_sb")
            nc.gpsimd.dma_start(wq_sb[:], wqT[:])
            wv_sb = wpool.tile([128, ECH, GF], MDT, name="wv_sb")
            nc.gpsimd.dma_start(wv_sb[:], wvT[:])
            # small constants on scalar: they finish before the first exp,
            # after which the scalar queue must stay DMA-free (descriptor
            # generation on the ACT sequencer slows every exp dispatch).
            bq_sb = wpool.tile([128, 1], F32, name="bq_sb")
            nc.scalar.dma_start(bq_sb[:], bq_d[:])
            kmask_sb = wpool.tile([128, GH], F32, name="kmask_sb")
            nc.scalar.dma_start(kmask_sb[:], kmask_d[:])
            kbm_sb = wpool.tile([128, GH], F32, name="kbm_sb")
            nc.scalar.dma_start(kbm_sb[:], kbm_d[:])
            bv_sb = wpool.tile([128, TB], F32, name="bv_sb")
            nc.scalar.dma_start(bv_sb[:], bv_d[:])

            xt = {}  # (name, b, t) -> sbuf tile (or [halfA, halfB])

            def load_x(name, b, t, eng, split=False):
                src = {"q": xq_d, "k": xk_d, "v": xv_d}[name]
                blk = b * NBLK + t
                if split:
                    # two e-halves so the projection can start on the
                    # first 4 contraction chunks while the rest lands
                    halves = []
                    for i in range(2):
                        tl = xpool.tile([128, ECH // 2, TB], MDT, tag="x",
                                        name=f"x{name}{b}{t}_{i}")
                        eng.dma_start(tl[:], src[:, blk, 4 * i:4 * i + 4])
                        halves.append(tl)
                    xt[(name, b, t)] = halves
                else:
                    tl = xpool.tile([128, ECH, TB], MDT, tag="x",
                                    name=f"x{name}{b}{t}")
                    eng.dma_start(tl[:], src[:, blk])
                    xt[(name, b, t)] = tl

            def xchunk(name, b, t, e):
                tl = xt[(name, b, t)]
                if isinstance(tl, list):
                    return tl[e // 4][:, e % 4, :]
                return tl[:, e, :]

            # DMA behavior (HW-measured): each engine's transfers run
            # roughly serially in issue order; concurrent transfers
            # fair-share the aggregate. So the whole exp-critical
            # sequence (K+Q, interleaved in consumption order, block-0
            # tiles halved) goes on the fast sync column; V rides
            # scalar/gpsimd in need order behind the weights.
            load_x("k", 0, 0, nc.sync, split=True)
            load_x("q", 0, 0, nc.sync, split=True)
            load_x("v", 0, 0, nc.gpsimd)
            for t in range(1, NBLK):
                load_x("k", 0, t, nc.sync)
                load_x("q", 0, t, nc.sync)
                load_x("v", 0, t, nc.scalar if t == 1 else nc.gpsimd)

            # ---- long-lived state ----
            QT = state.tile([128, TS], MDT, name="QT")
            AT = state.tile([128, TS], MDT, name="AT")
            # per-head zero-padded KT: rows [64h, 64h+64) hold head h's
            # K features, the other 64 rows stay zero -> scores matmuls
            # run K=128 (2x faster than K=64) with unmasked QT as rhs.
            KTp = [state.tile([128, TS], MDT, name=f"KTp{h}")
                   for h in range(GH)]

            # V: [128 tok, tok-chunk, head, 65]; col 64 = ones
            VT = state.tile([128, B * NKT, GH, HD + 1], MDT, name="VT")
            nc.gpsimd.memset(VT[:, :, :, HD:HD + 1], 1.0)

            ao = [state.tile([128, ECH, 2 * OC], MDT, name=f"ao{b}")
                  for b in range(B)]
            # one exchange per (head-parity, batch, 1024-token half):
            # 8 collectives of 256KB. More than that saturates the CC
            # engine (~10us fixed cost per collective, HW-measured).
            a2a_in = {}
            a2a_out = {}
            for h in range(GH):
                for b in range(B):
                    for hf in range(2):
                        a2a_in[(h, b, hf)] = dramp.tile(
                            [N_CORES, HD, OC], MDT, name=f"a2a_in{h}{b}{hf}")
                        a2a_out[(h, b, hf)] = dramp.tile(
                            [N_CORES, HD, OC], MDT, name=f"a2a_out{h}{b}{hf}")

            # ---- emission helpers ----
            def emit_qk_proj(name, b, t):
                w_sb = wq_sb if name == "q" else wk_sb
                col = b * S + t * TB
                csl = slice(col, col + TB)
                ps = ps_proj.tile([128, TB], F32, tag="pp",
                                  name=f"ps{name}{b}{t}")
                for e in range(ECH):
                    nc.tensor.matmul(ps[:], w_sb[:, e, :],
                                     xchunk(name, b, t, e),
                                     start=(e == 0), stop=(e == ECH - 1))
                if name == "q":
                    nc.vector.tensor_scalar_add(QT[:, csl], ps[:], bq_sb[:])
                else:
                    for h in range(GH):
                        nc.vector.tensor_scalar(
                            KTp[h][:, csl], ps[:],
                            kmask_sb[:, h:h + 1], kbm_sb[:, h:h + 1],
                            op0=mybir.AluOpType.mult,
                            op1=mybir.AluOpType.add)

            def emit_v_proj(b, t):
                psv = ps_proj.tile([128, TB], F32, tag="pp",
                                   name=f"psv{b}{t}")
                for e in range(ECH):
                    for m in range(4):
                        # NOTE: start=True clears has_written for the
                        # WHOLE psum bank, so only the very first matmul
                        # into this bank may set it.
                        nc.tensor.matmul(
                            psv[:, m * GF:(m + 1) * GF],
                            xchunk("v", b, t, e)[:, m * 128:(m + 1) * 128],
                            wv_sb[:, e, :],
                            start=(e == 0 and m == 0),
                            stop=(e == ECH - 1 and m == 3))
                kt0 = b * NKT + t * 4
                nc.vector.tensor_add(
                    VT[:, kt0:kt0 + 4, :, 0:HD],
                    psv[:].rearrange("p (m h d) -> p m h d", m=4, h=GH),
                    bv_sb[:].rearrange("p (m h d) -> p m h d", m=4, h=GH))

            def emit_attn(h, b, stage_eng):
                """Attention for head-parity h, batch b, processed
                kp-major over qb PAIRS: exps for (k-block, q-block)
                combinations are emitted in data-arrival order, so the
                exp stream ramps with the DMA instead of stalling on the
                full K range. After each pair (a 1024-token half):
                normalize + stage + AllToAll + receive into ao."""
                off = HD * h
                for qp in range(2):
                    qbs = (2 * qp, 2 * qp + 1)
                    pa = {}
                    exps = {qb: [] for qb in qbs}
                    for qb in qbs:
                        pa[qb] = ps_at.tile([HD + 1, TB], F32, tag="at",
                                            name=f"pa{h}{b}{qb}")
                    for kp in range(NP):
                        for qb in qbs:
                            qcol = b * S + qb * TB
                            pssc = ps_sc.tile([128, 2 * TB], F32, tag="sc",
                                              name=f"pssc{h}{b}{qb}{kp}")
                            for i in range(2):
                                kcol = b * S + (2 * kp + i) * 128
                                nc.tensor.matmul(
                                    pssc[:, i * TB:(i + 1) * TB],
                                    KTp[h][:, kcol:kcol + 128],
                                    QT[:, qcol:qcol + TB],
                                    start=True, stop=True)
                            ex = expp.tile([128, 2 * TB], MDT, tag="exp",
                                           name=f"ex{h}{b}{qb}{kp}")
                            nc.scalar.activation(ex[:], pssc[:], Act.Exp,
                                                 scale=0.125)
                            exps[qb].append(ex)
                        if kp >= 1:
                            for qb in qbs:
                                for i in range(2):
                                    kt = 2 * (kp - 1) + i
                                    nc.tensor.matmul(
                                        pa[qb][:],
                                        VT[:, b * NKT + kt, h, :],
                                        exps[qb][kp - 1][:,
                                                         i * TB:(i + 1) * TB],
                                        start=(kt == 0), stop=False)
                    for qb in qbs:
                        qcol = b * S + qb * TB
                        for i in range(2):
                            kt = 2 * (NP - 1) + i
                            nc.tensor.matmul(
                                pa[qb][:], VT[:, b * NKT + kt, h, :],
                                exps[qb][NP - 1][:, i * TB:(i + 1) * TB],
                                start=False, stop=(i == 1))
                        # normalize: attnT_h *= 1/den (broadcast over d)
                        dn = small.tile([1, TB], F32, tag="rc",
                                        name=f"dn{h}{b}{qb}")
                        nc.vector.tensor_copy(dn[:], pa[qb][HD:HD + 1, :])
                        rc1 = small.tile([1, TB], F32, tag="rc1",
                                         name=f"rc1{h}{b}{qb}")
                        nc.vector.reciprocal_approx_fast(rc1[:], dn[:])
                        rc = small.tile([HD, TB], F32, tag="rc2",
                                        name=f"rc{h}{b}{qb}")
                        nc.gpsimd.partition_broadcast(rc[:], rc1[:])
                        nc.vector.tensor_mul(
                            AT[off:off + HD, qcol:qcol + TB],
                            pa[qb][0:HD, :], rc[:])
                    key = (h, b, qp)
                    stage_eng.dma_start(
                        a2a_in[key][:].rearrange("j p n -> p j n"),
                        AT[off:off + HD,
                           b * S + 1024 * qp:b * S + 1024 * (qp + 1)]
                        .rearrange("p (j n) -> p j n", j=N_CORES))
                    nc.gpsimd.collective_compute(
                        "AllToAll",
                        mybir.AluOpType.bypass,
                        replica_groups=groups,
                        ins=[a2a_in[key][:]],
                        outs=[a2a_out[key][:]],
                    )
                    stage_eng.dma_start(
                        ao[b][off:off + HD, :, OC * qp:OC * (qp + 1)],
                        a2a_out[key][:].rearrange("j p n -> p j n"))

            def emit_out_proj(b, m, eng):
                ot = opool.tile([128, D], F32, tag="ot", name=f"ot{b}{m}")
                for fb in range(2):
                    fsl = slice(fb * 512, (fb + 1) * 512)
                    pso = ps_proj.tile([128, 512], F32, tag="pp",
                                       name=f"pso{b}{m}{fb}")
                    for e in range(ECH):
                        nc.tensor.matmul(
                            pso[:], ao[b][:, e, m * OC:(m + 1) * OC],
                            wo_sb[:, e, fsl],
                            start=(e == 0), stop=(e == ECH - 1))
                    nc.vector.tensor_add(ot[:, fsl], pso[:], bo_sb[:, fsl])
                    # per-half store so the last 256KB isn't serialized
                    # behind the second half's matmuls
                    eng.dma_start(out_d[b, m, :, fsl], ot[:, fsl])

            # ---- schedule ----
            # Emission order defines BOTH the RAW dependencies (a reader
            # must be emitted after its writer: the tracker doesn't know
            # about future writes) and the scheduler priority. So all of
            # a batch's projections are emitted before its attention, and
            # the attention is wrapped in high_priority() so its scores
            # preempt remaining projection work once their deps resolve
            # (keeps the ScalarE exp stream dense).
            for t in range(NBLK):
                emit_qk_proj("k", 0, t)
                emit_qk_proj("q", 0, t)
                emit_v_proj(0, t)

            # wo/bo are needed only from ~150us; they ride the sync column
            # behind the batch-0 K/Q loads.
            wo_sb = wpool.tile([128, ECH, D], MDT, name="wo_sb")
            nc.sync.dma_start(wo_sb[:], woT[:])
            bo_sb = wpool.tile([128, D], F32, name="bo_sb")
            nc.sync.dma_start(bo_sb[:], bo_d[:])

            with tc.high_priority():
                emit_attn(0, 0, nc.sync)
            emit_attn(1, 0, nc.gpsimd)

            # batch-1 x prefetch + projections (fill PE under batch-0
            # attention, ahead of the mid-kernel collectives)
            for t in range(NBLK):
                load_x("k", 1, t, nc.sync)
                load_x("q", 1, t, nc.sync)
                load_x("v", 1, t, nc.gpsimd)
            emit_qk_proj("k", 1, 0)
            emit_qk_proj("q", 1, 0)
            emit_v_proj(1, 0)
            for t in range(1, NBLK):
                emit_qk_proj("k", 1, t)
                emit_qk_proj("q", 1, t)
                emit_v_proj(1, t)

            # offset ~= instruction count of the batch-1 projections, so
            # batch-1 scores outrank leftover projection work but stay
            # after the batch-0 attention priorities.
            with tc.high_priority(offset=400):
                emit_attn(0, 1, nc.sync)
            emit_out_proj(0, 0, nc.sync)   # overlaps batch-1 attention
            emit_out_proj(0, 1, nc.sync)
            with tc.high_priority(offset=450):
                emit_attn(1, 1, nc.gpsimd)
            emit_out_proj(1, 0, nc.gpsimd)  # ready mid-phase: keeps PE warm
            # discarded matmuls: hold the PE p-state up while the last
            # AllToAll is in flight so the final projection runs at full
            # clock (idle >0.1us drops PE to the 1.2GHz mid state)
            warm = ps_at.tile([HD + 1, TB], F32, tag="at", name="warm")
            for _ in range(24):
                nc.tensor.matmul(warm[:], VT[:, 0, 0, :], QT[:, 0:TB],
                                 start=True, stop=True)
            emit_out_proj(1, 1, nc.gpsimd)  # the only tail work

    nc.compile()
    return nc


def _mm_np_dtype():
    if MM_DTYPE == "bf16":
        import ml_dtypes
        return np.dtype(ml_dtypes.bfloat16)
    return np.float32


def _prep_inputs(Q_input, K_input, V_input, Wq, bq, Wk, bk, Wv, bv, Wo, bo):
    """Build the 8 per-core input maps (host-side sharding + transposes)."""
    f32 = np.float32
    mmdt = _mm_np_dtype()
    xT = {}
    for nm, x in (("xq", Q_input), ("xk", K_input), ("xv", V_input)):
        x = np.asarray(x, f32).reshape(TS, D)
        # [tok, feat] -> [p, blk, e, n] with tok = blk*TB + n,
        # feat = e*128 + p  (8KB contiguous per (p, blk))
        xT[nm] = np.ascontiguousarray(
            x.reshape(B * NBLK, TB, ECH, 128).transpose(3, 0, 2, 1)
            .astype(mmdt))
    Wq, Wk, Wv, Wo = (np.asarray(w, f32) for w in (Wq, Wk, Wv, Wo))
    bq, bk, bv, bo = (np.asarray(v, f32) for v in (bq, bk, bv, bo))

    def peF(wT):  # [D, F] -> [128, ECH, F] partition-major (fat descriptors)
        return np.ascontiguousarray(
            wT.reshape(ECH, 128, wT.shape[1]).transpose(1, 0, 2).astype(mmdt))

    woT_full = peF(Wo.T)
    bo_bc = np.ascontiguousarray(np.broadcast_to(bo, (128, D)))
    kmask = np.zeros((128, GH), f32)
    for h in range(GH):
        kmask[HD * h:HD * h + HD, h] = 1.0

    in_maps = []
    for c in range(N_CORES):
        hsl = slice(c * GF, (c + 1) * GF)
        in_maps.append({
            **xT,
            "wqT": peF(Wq[hsl, :].T),
            "wkT": peF(Wk[hsl, :].T),
            "wvT": peF(Wv[hsl, :].T),
            "woT": woT_full,
            "bq": np.ascontiguousarray(bq[hsl].reshape(128, 1)),
            "bk": np.ascontiguousarray(bk[hsl].reshape(128, 1)),
            "kmask": kmask,
            "kbm": np.ascontiguousarray(kmask * bk[hsl].reshape(128, 1)),
            "bv": np.ascontiguousarray(
                np.broadcast_to(np.tile(bv[hsl], 4), (128, TB))),
            "bo": bo_bc,
        })
    return in_maps


def kernel(**inputs):
    from concourse.bass_utils import run_bass_kernel_spmd

    if "nc" not in _CACHE:
        _CACHE["nc"] = _build()
    nc = _CACHE["nc"]

    in_maps = _prep_inputs(**inputs)
    res = run_bass_kernel_spmd(nc, in_maps, core_ids=list(range(N_CORES)))

    out = np.empty((B, S, D), np.float32)
    for c in range(N_CORES):
        o = res.results[c]["out"]  # [B, 2, 128, D]
        for b in range(B):
            for m in range(2):
                t0 = 1024 * m + OC * c
                out[b, t0:t0 + OC, :] = o[b, m]
    return out
